# revision 27
# baseline (speedup 1.0000x reference)
"""KAN layer (polynomial basis) TRN2 kernel.

out = gelu(sum_{i,k} x[b,i]^k * W[i,k,j] + bias[j]),  exact gelu.
B=4096, D=1024, K=5, U=1024, fp32 I/O.

Strategy:
  - Data-parallel over batch: 8 cores x 512 rows each.
  - k=0 term (x^0=1) constant-folded on host into the bias.
  - Mixed precision chosen against the 2e-2 rel-err gate: k=1,2,3
    terms in fp8e4 with DoubleRow perf mode (2 contraction chunks per
    213.6ns matmul slot), k=4 term in bf16 (its variance share is 85%,
    fp8 would blow the gate).  Power-of-2 scales keep products exact:
    x/4 vs 4*W1, x^2/16 vs 16*W2, x^3/32 vs 32*W3.
  - Two-phase matmul stream: ALL fp8-DR groups (pairs x k=1..3 x u)
    first, then ALL bf16 k=4 groups u-major.  A bf16->fp8-DR mode
    switch costs ~+187ns on the PE; this layout pays it once instead
    of once per pair / once per u.  Phase B's per-u gelu + output DMA
    overlap the remaining bf16 matmuls, so only the last u's
    activation+store trail the stream.
  - Power computation split across engines so neither gates the PE:
    DVE does xq0 = x/4 (tensor_scalar), x2f = x*x, xq2 = (x2f/32)*x;
    ACT does xq1 = Square(x/4) and x4 = Square(x2f) (exact poly).
  - All DMAs (inputs AND outputs) ride the Sync-engine HW DGE queue in
    exact consumption order; the SW DGE stays unused, which drops 8
    DMA semaphores from the tile-context teardown drain.
  - x is transferred as fp16 (host cast, numerically free next to the
    fp8 terms): halving the x bytes keeps the ramping DMA pipe ahead
    of phase A's consumption, removing mid-stream weight stalls.
  - 22 fp8 warm-up matmuls (zeroed operand) bridge the ~5us from the
    post-preamble barrier to DMA readiness: the PE clock ramp needs
    ~4.2us of continuous busy to reach 2.4GHz, and an idle gap >2us
    resets it.
  - Output computed transposed ([U, B_local]) so the per-unit bias is
    a per-partition scalar fused into the Gelu; host transposes back.
"""

import os
import numpy as np
import ml_dtypes

from concourse import bacc
import concourse.mybir as mybir
import concourse.tile as tile
from concourse.bass_utils import run_bass_kernel_spmd

F32 = mybir.dt.float32
F16 = mybir.dt.float16
BF16 = mybir.dt.bfloat16
FP8 = mybir.dt.float8e4
AF = mybir.ActivationFunctionType
DR = mybir.MatmulPerfMode.DoubleRow
MUL = mybir.AluOpType.mult

NCORES = 8
B, D, K, U = 4096, 1024, 5, 1024
BL = B // NCORES  # 512 batch rows per core
ND = D // 128  # 8 d chunks
NP = ND // 2  # 4 d-chunk pairs
NU = U // 128  # 8 u chunks

XS = [0.25, 0.0625, 0.03125]  # fp8 pre-scales for x^1, x^2, x^3

LAST_EXEC_TIME_NS = None


def _build():
    nc = bacc.Bacc("TRN2", target_bir_lowering=False, debug=False)
    xt = nc.dram_tensor("xt", [D, BL], F16, kind="ExternalInput").ap()
    # wb16[d, p, m] = W[d*128+p, 4, m]
    wb16 = nc.dram_tensor(
        "wb16", [ND, 128, NU * 128], BF16, kind="ExternalInput"
    ).ap()
    # wf8[j, p, kk, c, m] = W[(2j+c)*128+p, kk+1, m] / XS[kk]
    wf8 = nc.dram_tensor(
        "wf8", [NP, 128, 3, 2, NU * 128], FP8, kind="ExternalInput"
    ).ap()
    bias2d = nc.dram_tensor("bias2d", [128, NU], F32, kind="ExternalInput").ap()
    out_t = nc.dram_tensor("out_t", [U, BL], F32, kind="ExternalOutput").ap()

    with tile.TileContext(nc) as tc:
        with (
            tc.tile_pool(name="const", bufs=1) as constp,
            tc.tile_pool(name="xp", bufs=8) as xp,
            tc.tile_pool(name="x2p", bufs=2) as x2p,
            tc.tile_pool(name="pw", bufs=4) as pw,
            tc.tile_pool(name="qp", bufs=3) as qp,
            tc.tile_pool(name="wp", bufs=4) as wp,
            tc.tile_pool(name="wq", bufs=4) as wq,
            tc.tile_pool(name="op", bufs=4) as op,
            tc.tile_pool(name="ps", bufs=1, space="PSUM") as ps,
        ):
            bias_sb = constp.tile([128, NU], F32, name="bias_sb")

            # 8 persistent PSUM accumulators, one bank per u chunk.
            pacc = [ps.tile([128, BL], F32, name=f"pacc{u}") for u in range(NU)]

            # Warm-up: fp8-DR dummies (same mode as the first real group,
            # avoiding the bf16->DR switch penalty), sized to end right
            # when the first real matmul's DMAs land (~11.4us).  The warm
            # memset rides the otherwise-empty GpSimd queue so the PE is
            # busy as early as possible after the preamble barrier.
            # Slim warm tile (512B/partition): the DVE memset is on the
            # first-matmul critical path, and a [128,2,256] fp8 memset is
            # ~2x faster than [128,2,512].
            warm = constp.tile([128, 2, 256], FP8, name="warm")
            nc.vector.memset(warm, 0)
            for _ in range(22):
                nc.tensor.matmul(
                    pacc[0][:, 0:256], warm[:, :, 0:128], warm,
                    start=True, stop=True, perf_mode=DR,
                )

            # ---- all input DMAs, kicked upfront in consumption order ----
            # Order = consumption order: pair-0 x first (the fp16-x ->
            # DVE xq0 -> first-matmul chain is the critical path; the
            # k=1 weight halves' completion sems land in its shadow),
            # then each pair's fp8 weights followed by the next pair's x.
            xfs = {}
            wqt = {}  # (j, kk) -> tile or (0,0) -> (half_a, half_b)
            wbt = {}

            def kick_x(j):
                for c in range(2):
                    t = xp.tile([128, BL], F16, name="xf", tag="xf")
                    nc.sync.dma_start(
                        t, xt[(2 * j + c) * 128 : (2 * j + c + 1) * 128, :]
                    )
                    xfs[j, c] = t

            def kick_wq(j, kk):
                t = wq.tile(
                    [128, 2, NU * 128], FP8, name=f"wq{kk}",
                    tag=f"wq{kk}" if kk else "wq0",
                )
                nc.sync.dma_start(t, wf8[j][:, kk])
                wqt[j, kk] = t

            for c in range(2):
                t = xp.tile([128, BL], F16, name="xf", tag="xf")
                nc.sync.dma_start(t, xt[c * 128 : (c + 1) * 128, :])
                xfs[0, c] = t
            wq0a = wq.tile([128, 2, 4 * 128], FP8, name="wq0a", tag="wq0a")
            wq0b = wq.tile([128, 2, 4 * 128], FP8, name="wq0b", tag="wq0b")
            nc.sync.dma_start(wq0a, wf8[0][:, 0, :, 0 : 4 * 128])
            nc.sync.dma_start(wq0b, wf8[0][:, 0, :, 4 * 128 : NU * 128])
            wqt[0, 0] = (wq0a, wq0b)
            kick_wq(0, 1)
            kick_wq(0, 2)
            for j in range(1, NP):
                kick_x(j)
                for kk in range(3):
                    kick_wq(j, kk)
            # k=4 bf16 weights (phase B) and bias last
            for j in range(NP):
                d0 = 2 * j
                for c in range(2):
                    t = wp.tile([128, NU * 128], BF16, name="wb", tag=f"wb{c}")
                    nc.sync.dma_start(t, wb16[d0 + c])
                    wbt[j, c] = t
            nc.sync.dma_start(bias_sb, bias2d)

            # ---- power computation: DVE + ACT split ----
            xq = {}   # (j, kk) -> fp8 [128, 2, BL]; (0,0) -> (A, B) halves
            x4t = {}  # (j, c) -> bf16 [128, BL]
            for j in range(NP):
                x2q = qp.tile([128, 2, BL], FP8, name="x2q", tag="x2q")
                x3q = qp.tile([128, 2, BL], FP8, name="x3q", tag="x3q")
                xq[j, 1], xq[j, 2] = x2q, x3q
                x2fs = [
                    x2p.tile([128, BL], F32, name="x2f", tag=f"x2f{c}")
                    for c in range(2)
                ]
                x1q = qp.tile([128, 2, BL], FP8, name="x1q", tag="x1q")
                xq[j, 0] = x1q
                for c in range(2):
                    nc.vector.tensor_scalar_mul(x1q[:, c], xfs[j, c], XS[0])
                for c in range(2):
                    # ACT: x^2/16 = Square(x/4)
                    nc.scalar.activation(
                        x2q[:, c], xfs[j, c], AF.Square, scale=XS[0]
                    )
                for c in range(2):
                    nc.vector.tensor_mul(
                        out=x2fs[c], in0=xfs[j, c], in1=xfs[j, c]
                    )
                for c in range(2):
                    nc.vector.scalar_tensor_tensor(
                        x3q[:, c], x2fs[c], XS[2], xfs[j, c],
                        op0=MUL, op1=MUL,
                    )
                for c in range(2):
                    # ACT: x^4 = Square(x^2)
                    x4 = pw.tile([128, BL], BF16, name="x4b", tag=f"x4b{c}")
                    nc.scalar.activation(x4, x2fs[c], AF.Square)
                    x4t[j, c] = x4

            # ---- phase A: all fp8 DoubleRow groups (k=1,2,3) ----
            for j in range(NP):
                for kk in range(3):
                    for u in range(NU):
                        us = slice(u * 128, (u + 1) * 128)
                        if j == 0 and kk == 0:
                            half = wqt[0, 0][u // 4]
                            lhsT = half[:, :, (u % 4) * 128 : (u % 4 + 1) * 128]
                        else:
                            lhsT = wqt[j, kk][:, :, us]
                        nc.tensor.matmul(
                            pacc[u], lhsT, xq[j, kk],
                            start=(j == 0 and kk == 0), stop=False,
                            perf_mode=DR,
                        )

            # ---- phase B: all bf16 k=4 groups, u-major, fused epilogue ----
            for u in range(NU):
                us = slice(u * 128, (u + 1) * 128)
                for j in range(NP):
                    for c in range(2):
                        nc.tensor.matmul(
                            pacc[u], wbt[j, c][:, us], x4t[j, c],
                            start=False, stop=(j == NP - 1 and c == 1),
                        )
                osb = op.tile([128, BL], F32, name="osb", tag="osb")
                nc.scalar.activation(
                    osb, pacc[u], AF.Gelu, bias=bias_sb[:, u : u + 1]
                )
                if u < NU - 1:
                    nc.sync.dma_start(out_t[u * 128 : (u + 1) * 128, :], osb)
                else:
                    # Final store split by partition rows: the exit barrier
                    # waits on this transfer's completion acks, which pace
                    # per 8-line descriptor batch — a 32-line final piece
                    # has a 4-batch ack window instead of 16, and the
                    # 96-line piece's acks overlap the final piece's data.
                    nc.sync.dma_start(
                        out_t[u * 128 : u * 128 + 96, :], osb[0:96]
                    )
                    nc.sync.dma_start(
                        out_t[u * 128 + 96 : (u + 1) * 128, :], osb[96:128]
                    )

    nc.compile()
    return nc


_NC_CACHE = None


def kernel(x, basis_weights, bias):
    global _NC_CACHE, LAST_EXEC_TIME_NS
    x = np.asarray(x, dtype=np.float32)
    W = np.asarray(basis_weights, dtype=np.float32)
    bias = np.asarray(bias, dtype=np.float32)

    # ---- host prep (layout only + constant folding of the x^0 term) ----
    xT = np.ascontiguousarray(x.T).astype(np.float16)  # (D, B), fp16 transfer
    wb16 = np.ascontiguousarray(
        W[:, 4, :].reshape(ND, 128, NU * 128)
    ).astype(ml_dtypes.bfloat16)
    wk = W[:, 1:4, :].reshape(NP, 2, 128, 3, NU * 128).transpose(0, 2, 3, 1, 4)
    wk = wk * (1.0 / np.array(XS, dtype=np.float32)).reshape(1, 1, 3, 1, 1)
    wf8 = np.ascontiguousarray(np.clip(wk, -240.0, 240.0)).astype(
        ml_dtypes.float8_e4m3
    )  # [NP, 128, 3, 2, NU*128]
    bias_total = (
        bias.astype(np.float64) + W[:, 0, :].astype(np.float64).sum(axis=0)
    ).astype(np.float32)
    bias2d = np.ascontiguousarray(bias_total.reshape(NU, 128).T)

    in_maps = []
    for i in range(NCORES):
        xt_i = np.ascontiguousarray(xT[:, i * BL : (i + 1) * BL])
        in_maps.append(
            {"xt": xt_i, "wb16": wb16, "wf8": wf8, "bias2d": bias2d}
        )

    if _NC_CACHE is None:
        _NC_CACHE = _build()
    nc = _NC_CACHE

    trace = bool(os.environ.get("KERNEL_TRACE"))
    res = run_bass_kernel_spmd(
        nc, in_maps, core_ids=list(range(NCORES)), trace=trace
    )
    LAST_EXEC_TIME_NS = res.exec_time_ns

    out = np.empty((B, U), dtype=np.float32)
    for i in range(NCORES):
        out[i * BL : (i + 1) * BL, :] = res.results[i]["out_t"].T
    return out


# revision 29
# speedup vs baseline: 1.0139x; 1.0139x over previous
"""KAN layer (polynomial basis) TRN2 kernel.

out = gelu(sum_{i,k} x[b,i]^k * W[i,k,j] + bias[j]),  exact gelu.
B=4096, D=1024, K=5, U=1024, fp32 I/O.

Strategy:
  - Data-parallel over batch: 8 cores x 512 rows each.
  - k=0 term (x^0=1) constant-folded on host into the bias.
  - Mixed precision chosen against the 2e-2 rel-err gate: k=1,2,3
    terms in fp8e4 with DoubleRow perf mode (2 contraction chunks per
    213.6ns matmul slot), k=4 term in bf16 (its variance share is 85%,
    fp8 would blow the gate).  Power-of-2 scales keep products exact:
    x/4 vs 4*W1, x^2/16 vs 16*W2, x^3/32 vs 32*W3.
  - Two-phase matmul stream: ALL fp8-DR groups (pairs x k=1..3 x u)
    first, then ALL bf16 k=4 groups u-major.  A bf16->fp8-DR mode
    switch costs ~+187ns on the PE; this layout pays it once instead
    of once per pair / once per u.  Phase B's per-u gelu + output DMA
    overlap the remaining bf16 matmuls, so only the last u's
    activation+store trail the stream.
  - Power computation split across engines so neither gates the PE:
    DVE does xq0 = x/4 (tensor_scalar), x2f = x*x, xq2 = (x2f/32)*x;
    ACT does xq1 = Square(x/4) and x4 = Square(x2f) (exact poly).
  - All DMAs (inputs AND outputs) ride the Sync-engine HW DGE queue in
    exact consumption order; the SW DGE stays unused, which drops 8
    DMA semaphores from the tile-context teardown drain.
  - x is transferred as fp16 (host cast, numerically free next to the
    fp8 terms): halving the x bytes keeps the ramping DMA pipe ahead
    of phase A's consumption, removing mid-stream weight stalls.
  - 22 fp8 warm-up matmuls (zeroed operand) bridge the ~5us from the
    post-preamble barrier to DMA readiness: the PE clock ramp needs
    ~4.2us of continuous busy to reach 2.4GHz, and an idle gap >2us
    resets it.
  - Output computed transposed ([U, B_local]) so the per-unit bias is
    a per-partition scalar fused into the Gelu; host transposes back.
"""

import os
import numpy as np
import ml_dtypes

from concourse import bacc
import concourse.mybir as mybir
import concourse.tile as tile
from concourse.bass_utils import run_bass_kernel_spmd

F32 = mybir.dt.float32
F16 = mybir.dt.float16
BF16 = mybir.dt.bfloat16
FP8 = mybir.dt.float8e4
AF = mybir.ActivationFunctionType
DR = mybir.MatmulPerfMode.DoubleRow
MUL = mybir.AluOpType.mult

NCORES = 8
B, D, K, U = 4096, 1024, 5, 1024
BL = B // NCORES  # 512 batch rows per core
ND = D // 128  # 8 d chunks
NP = ND // 2  # 4 d-chunk pairs
NU = U // 128  # 8 u chunks

XS = [0.25, 0.0625, 0.03125]  # fp8 pre-scales for x^1, x^2, x^3

LAST_EXEC_TIME_NS = None


def _build():
    nc = bacc.Bacc("TRN2", target_bir_lowering=False, debug=False)
    xt = nc.dram_tensor("xt", [D, BL], F16, kind="ExternalInput").ap()
    # wb16[d, p, m] = W[d*128+p, 4, m]
    wb16 = nc.dram_tensor(
        "wb16", [ND, 128, NU * 128], BF16, kind="ExternalInput"
    ).ap()
    # wf8[j, p, kk, c, m] = W[(2j+c)*128+p, kk+1, m] / XS[kk]
    wf8 = nc.dram_tensor(
        "wf8", [NP, 128, 3, 2, NU * 128], FP8, kind="ExternalInput"
    ).ap()
    bias2d = nc.dram_tensor("bias2d", [128, NU], F32, kind="ExternalInput").ap()
    out_t = nc.dram_tensor("out_t", [U, BL], F32, kind="ExternalOutput").ap()

    with tile.TileContext(nc) as tc:
        with (
            tc.tile_pool(name="const", bufs=1) as constp,
            tc.tile_pool(name="xp", bufs=8) as xp,
            tc.tile_pool(name="x2p", bufs=3) as x2p,
            tc.tile_pool(name="pw", bufs=4) as pw,
            tc.tile_pool(name="qp", bufs=4) as qp,
            tc.tile_pool(name="wp", bufs=4) as wp,
            tc.tile_pool(name="wq", bufs=4) as wq,
            tc.tile_pool(name="op", bufs=8) as op,
            tc.tile_pool(name="ps", bufs=1, space="PSUM") as ps,
        ):
            bias_sb = constp.tile([128, NU], F32, name="bias_sb")

            # 8 persistent PSUM accumulators, one bank per u chunk.
            pacc = [ps.tile([128, BL], F32, name=f"pacc{u}") for u in range(NU)]

            # Warm-up: fp8-DR dummies (same mode as the first real group,
            # avoiding the bf16->DR switch penalty), sized to end right
            # when the first real matmul's DMAs land (~11.4us).  The warm
            # memset rides the otherwise-empty GpSimd queue so the PE is
            # busy as early as possible after the preamble barrier.
            # Slim warm tile (512B/partition): the DVE memset is on the
            # first-matmul critical path, and a [128,2,256] fp8 memset is
            # ~2x faster than [128,2,512].
            warm = constp.tile([128, 2, 256], FP8, name="warm")
            nc.vector.memset(warm, 0)
            for _ in range(22):
                nc.tensor.matmul(
                    pacc[0][:, 0:256], warm[:, :, 0:128], warm,
                    start=True, stop=True, perf_mode=DR,
                )

            # ---- all input DMAs, kicked upfront in consumption order ----
            # Order = consumption order: pair-0 x first (the fp16-x ->
            # DVE xq0 -> first-matmul chain is the critical path; the
            # k=1 weight halves' completion sems land in its shadow),
            # then each pair's fp8 weights followed by the next pair's x.
            xfs = {}
            wqt = {}  # (j, kk) -> tile or (0,0) -> (half_a, half_b)
            wbt = {}

            def kick_x(j):
                for c in range(2):
                    t = xp.tile([128, BL], F16, name="xf", tag="xf")
                    nc.sync.dma_start(
                        t, xt[(2 * j + c) * 128 : (2 * j + c + 1) * 128, :]
                    )
                    xfs[j, c] = t

            def kick_wq(j, kk):
                t = wq.tile(
                    [128, 2, NU * 128], FP8, name=f"wq{kk}",
                    tag=f"wq{kk}" if kk else "wq0",
                )
                nc.sync.dma_start(t, wf8[j][:, kk])
                wqt[j, kk] = t

            for c in range(2):
                t = xp.tile([128, BL], F16, name="xf", tag="xf")
                nc.sync.dma_start(t, xt[c * 128 : (c + 1) * 128, :])
                xfs[0, c] = t
            wq0a = wq.tile([128, 2, 4 * 128], FP8, name="wq0a", tag="wq0a")
            wq0b = wq.tile([128, 2, 4 * 128], FP8, name="wq0b", tag="wq0b")
            nc.sync.dma_start(wq0a, wf8[0][:, 0, :, 0 : 4 * 128])
            nc.sync.dma_start(wq0b, wf8[0][:, 0, :, 4 * 128 : NU * 128])
            wqt[0, 0] = (wq0a, wq0b)
            kick_wq(0, 1)
            kick_wq(0, 2)
            for j in range(1, NP):
                kick_x(j)
                for kk in range(3):
                    kick_wq(j, kk)
            # k=4 bf16 weights (phase B) and bias last
            for j in range(NP):
                d0 = 2 * j
                for c in range(2):
                    t = wp.tile([128, NU * 128], BF16, name="wb", tag=f"wb{c}")
                    nc.sync.dma_start(t, wb16[d0 + c])
                    wbt[j, c] = t
            nc.sync.dma_start(bias_sb, bias2d)

            # ---- power computation: DVE + ACT split ----
            xq = {}   # (j, kk) -> fp8 [128, 2, BL]; (0,0) -> (A, B) halves
            x4t = {}  # (j, c) -> bf16 [128, BL]
            for j in range(NP):
                x2q = qp.tile([128, 2, BL], FP8, name="x2q", tag="x2q")
                x3q = qp.tile([128, 2, BL], FP8, name="x3q", tag="x3q")
                xq[j, 1], xq[j, 2] = x2q, x3q
                x2fs = [
                    x2p.tile([128, BL], F32, name="x2f", tag=f"x2f{c}")
                    for c in range(2)
                ]
                x1q = qp.tile([128, 2, BL], FP8, name="x1q", tag="x1q")
                xq[j, 0] = x1q
                for c in range(2):
                    nc.vector.tensor_scalar_mul(x1q[:, c], xfs[j, c], XS[0])
                for c in range(2):
                    # ACT: x^2/16 = Square(x/4)
                    nc.scalar.activation(
                        x2q[:, c], xfs[j, c], AF.Square, scale=XS[0]
                    )
                for c in range(2):
                    nc.vector.tensor_mul(
                        out=x2fs[c], in0=xfs[j, c], in1=xfs[j, c]
                    )
                for c in range(2):
                    nc.vector.scalar_tensor_tensor(
                        x3q[:, c], x2fs[c], XS[2], xfs[j, c],
                        op0=MUL, op1=MUL,
                    )
                for c in range(2):
                    # ACT: x^4 = Square(x^2)
                    x4 = pw.tile([128, BL], BF16, name="x4b", tag=f"x4b{c}")
                    nc.scalar.activation(x4, x2fs[c], AF.Square)
                    x4t[j, c] = x4

            # ---- phase A: all fp8 DoubleRow groups (k=1,2,3) ----
            for j in range(NP):
                for kk in range(3):
                    for u in range(NU):
                        us = slice(u * 128, (u + 1) * 128)
                        if j == 0 and kk == 0:
                            half = wqt[0, 0][u // 4]
                            lhsT = half[:, :, (u % 4) * 128 : (u % 4 + 1) * 128]
                        else:
                            lhsT = wqt[j, kk][:, :, us]
                        nc.tensor.matmul(
                            pacc[u], lhsT, xq[j, kk],
                            start=(j == 0 and kk == 0), stop=False,
                            perf_mode=DR,
                        )

            # ---- phase B: all bf16 k=4 groups, u-major, fused epilogue ----
            for u in range(NU):
                us = slice(u * 128, (u + 1) * 128)
                for j in range(NP):
                    for c in range(2):
                        nc.tensor.matmul(
                            pacc[u], wbt[j, c][:, us], x4t[j, c],
                            start=False, stop=(j == NP - 1 and c == 1),
                        )
                osb = op.tile([128, BL], F32, name="osb", tag="osb")
                nc.scalar.activation(
                    osb, pacc[u], AF.Gelu, bias=bias_sb[:, u : u + 1]
                )
                nc.sync.dma_start(out_t[u * 128 : (u + 1) * 128, :], osb)

    nc.compile()
    return nc


_NC_CACHE = None


def kernel(x, basis_weights, bias):
    global _NC_CACHE, LAST_EXEC_TIME_NS
    x = np.asarray(x, dtype=np.float32)
    W = np.asarray(basis_weights, dtype=np.float32)
    bias = np.asarray(bias, dtype=np.float32)

    # ---- host prep (layout only + constant folding of the x^0 term) ----
    xT = np.ascontiguousarray(x.T).astype(np.float16)  # (D, B), fp16 transfer
    wb16 = np.ascontiguousarray(
        W[:, 4, :].reshape(ND, 128, NU * 128)
    ).astype(ml_dtypes.bfloat16)
    wk = W[:, 1:4, :].reshape(NP, 2, 128, 3, NU * 128).transpose(0, 2, 3, 1, 4)
    wk = wk * (1.0 / np.array(XS, dtype=np.float32)).reshape(1, 1, 3, 1, 1)
    wf8 = np.ascontiguousarray(np.clip(wk, -240.0, 240.0)).astype(
        ml_dtypes.float8_e4m3
    )  # [NP, 128, 3, 2, NU*128]
    bias_total = (
        bias.astype(np.float64) + W[:, 0, :].astype(np.float64).sum(axis=0)
    ).astype(np.float32)
    bias2d = np.ascontiguousarray(bias_total.reshape(NU, 128).T)

    in_maps = []
    for i in range(NCORES):
        xt_i = np.ascontiguousarray(xT[:, i * BL : (i + 1) * BL])
        in_maps.append(
            {"xt": xt_i, "wb16": wb16, "wf8": wf8, "bias2d": bias2d}
        )

    if _NC_CACHE is None:
        _NC_CACHE = _build()
    nc = _NC_CACHE

    trace = bool(os.environ.get("KERNEL_TRACE"))
    res = run_bass_kernel_spmd(
        nc, in_maps, core_ids=list(range(NCORES)), trace=trace
    )
    LAST_EXEC_TIME_NS = res.exec_time_ns

    out = np.empty((B, U), dtype=np.float32)
    for i in range(NCORES):
        out[i * BL : (i + 1) * BL, :] = res.results[i]["out_t"].T
    return out
